# revision 21
# baseline (speedup 1.0000x reference)
"""Trainium2 Bass kernel for nn_FCAutoEncoder (ragged_sequence).

Strategy:
  * Linear-linear boundaries fold on the host (exact, fp32):
      - per-size input scaler (s_k->1008) + encoder L1 (1008->512)
        -> C_k = We1 @ Win_k  (s_k->512), bias We1 @ bin_k + be1
      - encoder L3 (256->128) + decoder D1 (128->256)
        -> M = Wd1 @ We3      (256->256), bias Wd1 @ be3 + bd1
      - decoder D3 (512->1008) + per-size output scaler (1008->s_k)
        -> G_k = Wout_k @ Wd3 (512->s_k), bias Wout_k @ bd3 + bout_k
    Device work per column: s_k->512->256->256->512->s_k (~1/3 of the
    unfolded FLOPs).
  * Host: bucket rows by seq_length (5 sizes), split each bucket evenly
    across 8 cores (pure data parallel), transpose to feature-major.
  * bf16 everywhere with fp32 PSUM accumulation (rel err ~4e-3 vs fp32
    reference, tolerance 2e-2); LDWEIGHTS (~112ns) hides under the
    ~171ns row stream per matmul.
  * All weights live in ONE packed dram blob, DMAed in a handful of
    slices in first-use order on the Sync queue (per-DMA issue costs
    ~700ns of sequencer time, so fewer/bigger is better); per-unit x
    is one batched DMA, prefetched a unit ahead, also on Sync, AFTER
    the startup-critical weight slices.
  * Unit order [3, 4, 2, 1, 0]: a small bucket first so the PE starts
    on ~0.7MB of DMA instead of ~1.9MB; bucket 4's big weights stream
    during unit 0's compute (cin4 in 4 column-slices so unit 1's
    M-outer matmuls start on the first slice).
  * Two-unit software pipeline: decoder tail of unit i interleaves into
    the encoder head of unit i+1 so the PE never stalls (stalls also
    reset the PE DVFS ramp -> half-clock matmuls).
  * Host: transpose back, scatter rows to original order.
"""
import os
import sys

sys.path.insert(0, "/opt/trn_rl_repo")

import numpy as np
import ml_dtypes

BF16 = ml_dtypes.bfloat16

SIZES = (36, 72, 144, 288, 1008)
SP = (128, 128, 256, 384, 1024)   # SIZES padded to multiples of 128
BASE = 1008
BASE_P = 1024
H1, H2 = 512, 256
N_CORES = 8
MAX_CHUNK = 512
ACT_BUFS = 33
UNIT_ORDER = (3, 4, 2, 1, 0)

_last_exec_ns = None
_prog_cache = {}


def _tiles(n, t=128):
    return [(s, min(t, n - s)) for s in range(0, n, t)]


def _chunks(c, maxn=MAX_CHUNK):
    if c <= 0:
        return []
    assert c % 2 == 0
    half = c // 2
    n = (c + maxn - 1) // maxn
    base, rem = divmod(half, n)
    out, off = [], 0
    for i in range(n):
        sz = 2 * (base + (1 if i < rem else 0))
        out.append((off, sz))
        off += sz
    return out


def _bias_layout():
    cols = []
    for k in range(5):
        for (js, jp) in _tiles(H1):
            cols.append(("L1", k, js, jp))
    for (js, jp) in _tiles(H2):
        cols.append(("L2", 0, js, jp))
    for (js, jp) in _tiles(H2):
        cols.append(("MID", 0, js, jp))
    for (js, jp) in _tiles(H1):
        cols.append(("D2", 0, js, jp))
    for k in range(5):
        for (os_, op) in _tiles(SIZES[k]):
            cols.append(("out", k, os_, op))
    return cols


def _blob_layout():
    """Packed weight blob: name -> (col_offset, n_ktiles, n_cols) in
    first-use order for UNIT_ORDER."""
    k0 = UNIT_ORDER[0]
    entries = [(f"cin{k0}", SP[k0] // 128, H1),
               ("we2", H1 // 128, H2),
               ("mid", H2 // 128, H2),
               ("wd2", H2 // 128, H1),
               (f"gout{k0}", H1 // 128, SIZES[k0])]
    for k in UNIT_ORDER[1:]:
        entries.append((f"cin{k}", SP[k] // 128, H1))
        entries.append((f"gout{k}", H1 // 128, SIZES[k]))
    lay = {}
    off = 0
    for name, t, c in entries:
        lay[name] = (off, t, c)
        off += t * c
    return lay, off


def _build_program(c_ks, R):
    import concourse.bacc as bacc
    import concourse.mybir as mybir
    from concourse import tile

    f32 = mybir.dt.float32
    bf16 = mybir.dt.bfloat16
    AF = mybir.ActivationFunctionType
    ALU = mybir.AluOpType

    bias_cols = _bias_layout()
    bias_idx = {c[:3]: i for i, c in enumerate(bias_cols)}

    def bcol(layer, k, start):
        return bias_idx[(layer, k, start)]

    blob_lay, blob_cols = _blob_layout()

    nc = bacc.Bacc(None, target_bir_lowering=False, debug=False, num_devices=1)

    xT = nc.dram_tensor("xT", [BASE_P, R], bf16, kind="ExternalInput").ap()
    outT = nc.dram_tensor("outT", [BASE, R], bf16, kind="ExternalOutput").ap()
    blobD = nc.dram_tensor("wblob", [128, blob_cols], bf16,
                           kind="ExternalInput").ap()
    biasD = nc.dram_tensor("biases", [128, len(bias_cols)], f32,
                           kind="ExternalInput").ap()

    with tile.TileContext(nc) as tc:
        with (
            tc.tile_pool(name="wp", bufs=1) as wp,
            tc.tile_pool(name="ap", bufs=ACT_BUFS) as apool,
            tc.tile_pool(name="xp", bufs=6) as xpool,
            tc.tile_pool(name="pp", bufs=8, space="PSUM") as pp,
        ):
            bias_t = wp.tile([128, len(bias_cols)], f32, tag="bias")
            blob = wp.tile([128, blob_cols], bf16, tag="blob")

            def wsl(name, i, js, jp):
                off, t, c = blob_lay[name]
                a = off + i * c + js
                return blob[:, a:a + jp]

            def evac(psum, mp, cn, bias_j, relu, eng, out_dt=bf16):
                o = apool.tile([mp, cn], out_dt, tag="act")
                b = bias_t[:mp, bias_j:bias_j + 1]
                if eng == "act":
                    nc.scalar.activation(
                        o[:], psum[:], AF.Relu if relu else AF.Identity, bias=b
                    )
                else:
                    if relu:
                        nc.vector.tensor_scalar(
                            o[:], psum[:], b, 0.0, ALU.add, ALU.max
                        )
                    else:
                        nc.vector.tensor_scalar_add(o[:], psum[:], b)
                return o[:]

            def sub_layer(in_tiles, wname, n_in, n_out, bias_layer, bias_k,
                          relu, cn, eng0=0):
                outs = []
                nkt = n_in // 128
                for mi, (js, jp) in enumerate(_tiles(n_out)):
                    psum = pp.tile([jp, cn], f32, tag="ps")
                    for i in range(nkt):
                        nc.tensor.matmul(
                            psum[:], wsl(wname, i, js, jp), in_tiles[i],
                            start=(i == 0), stop=(i == nkt - 1),
                        )
                    outs.append(
                        evac(psum, jp, cn, bcol(bias_layer, bias_k, js),
                             relu, "dve" if (mi + eng0) % 2 == 0 else "act")
                    )
                return outs

            def load_x(k, g0, cn):
                """One batched DMA on Sync: xT[0:SP_k, ...] -> [128,t,cn]."""
                t = SP[k] // 128
                tl = xpool.tile([128, t, cn], bf16, tag="xin")
                r = xT[0:SP[k], g0:g0 + cn].rearrange("(t p) c -> p t c",
                                                      p=128)
                nc.sync.dma_start(tl[:], r)
                return [tl[:, i, :] for i in range(t)]

            def emit_out(k, g0, cn, d2):
                for oi, (os_, op) in enumerate(_tiles(SIZES[k])):
                    psum = pp.tile([op, cn], f32, tag="ps")
                    for i in range(H1 // 128):
                        nc.tensor.matmul(
                            psum[:], wsl(f"gout{k}", i, os_, op), d2[i],
                            start=(i == 0), stop=(i == H1 // 128 - 1),
                        )
                    ot = evac(psum, op, cn, bcol("out", k, os_),
                              False, "act" if oi % 2 else "dve")
                    nc.sync.dma_start(
                        outT[os_:os_ + op, g0:g0 + cn], ot[:]
                    )

            def tail_stages(k, g0, cn, h2):
                m2 = sub_layer(h2, "mid", H2, H2, "MID", 0, True, cn, 1)
                yield
                d2 = sub_layer(m2, "wd2", H2, H1, "D2", 0, True, cn)
                yield
                emit_out(k, g0, cn, d2)

            offs = {}
            off = 0
            for k in range(5):
                offs[k] = off
                off += c_ks[k]

            units = []
            for k in UNIT_ORDER:
                for (c0, cn) in _chunks(c_ks[k]):
                    units.append((k, offs[k] + c0, cn))

            # ---- startup ----
            warm_a = wp.tile([1, 2], f32, tag="warm_a")
            warm_b = wp.tile([1, 2], f32, tag="warm_b")
            nc.any.memset(warm_a[:], 0.0)
            nc.scalar.activation(warm_b[:], warm_a[:], AF.Relu)

            k0, g00, cn0 = units[0]
            t0 = SP[k0] // 128
            # first unit's x: one tile per DMA for exact dependencies,
            # on the Scalar HWDGE queue (parallel to Sync's weight DMAs)
            h = (t0 + 1) // 2
            x0a = xpool.tile([128, h, cn0], bf16, tag="xin", name="x0a")
            x0b = None
            if t0 - h > 0:
                x0b = xpool.tile([128, t0 - h, cn0], bf16, tag="xin",
                                 name="x0b")
            x0r = xT[0:SP[k0], g00:g00 + cn0].rearrange("(t p) c -> p t c",
                                                        p=128)
            nc.scalar.dma_start(bias_t[:], biasD[:])
            nc.scalar.dma_start(x0a[:], x0r[:, 0:h, :])
            if x0b is not None:
                nc.scalar.dma_start(x0b[:], x0r[:, h:t0, :])
            xts0 = [x0a[:, i, :] for i in range(h)] + \
                   ([x0b[:, i, :] for i in range(t0 - h)] if x0b is not None
                    else [])

            # weight blob on Sync, in priority order:
            #  1. first unit's cin, sliced per K-plane (K-outer feed)
            #  2. shared mid layers + first unit's gout
            #  3. second unit's x (big bucket), then its cin in 4
            #     column-slices (M-outer feed) and its gout
            #  4. everything else, then remaining x prefetches
            def blob_dma(a, b):
                nc.sync.dma_start(blob[:, a:b], blobD[:, a:b])

            c0_off, c0_t, c0_c = blob_lay[f"cin{k0}"]
            for i in range(0, c0_t, 2):
                j = min(i + 2, c0_t)
                blob_dma(c0_off + i * c0_c, c0_off + j * c0_c)
            g0_off, g0_t, g0_c = blob_lay[f"gout{k0}"]
            blob_dma(c0_off + c0_t * c0_c, g0_off + g0_t * g0_c)

            xpre = {}
            if len(units) > 1:
                k1, g01, cn1 = units[1]
                xpre[(k1, g01)] = load_x(k1, g01, cn1)
                c1_off, c1_t, c1_c = blob_lay[f"cin{k1}"]
                c1r_s = blob[:, c1_off:c1_off + c1_t * c1_c].rearrange(
                    "p (t c) -> p t c", c=c1_c)
                c1r_d = blobD[:, c1_off:c1_off + c1_t * c1_c].rearrange(
                    "p (t c) -> p t c", c=c1_c)
                for cs in range(0, c1_c, 128):
                    ce = min(cs + 128, c1_c)
                    nc.sync.dma_start(c1r_s[:, :, cs:ce], c1r_d[:, :, cs:ce])
                g1_off, g1_t, g1_c = blob_lay[f"gout{k1}"]
                blob_dma(g1_off, g1_off + g1_t * g1_c)
                if g1_off + g1_t * g1_c < blob_cols:
                    blob_dma(g1_off + g1_t * g1_c, blob_cols)
            for (k, g0, cn) in units[2:]:
                xpre[(k, g0)] = load_x(k, g0, cn)

            tail_prev = None
            for ui, (k, g0, cn) in enumerate(units):
                first = ui == 0
                if first:
                    # K-outer so matmuls start while x/cin stream in
                    nkt = SP[k] // 128
                    jt = _tiles(H1)
                    psums = [pp.tile([jp, cn], f32, tag="ps",
                                     name=f"ps_s0_{mi}")
                             for mi, (js, jp) in enumerate(jt)]
                    for i in range(nkt):
                        for mi, (js, jp) in enumerate(jt):
                            nc.tensor.matmul(
                                psums[mi][:], wsl(f"cin{k}", i, js, jp),
                                xts0[i],
                                start=(i == 0), stop=(i == nkt - 1),
                                skip_group_check=True,
                            )
                    h1 = [
                        evac(psums[mi], jp, cn, bcol("L1", k, js), True,
                             "dve" if mi % 2 == 0 else "act")
                        for mi, (js, jp) in enumerate(jt)
                    ]
                else:
                    xts = xpre.pop((k, g0))
                    h1 = sub_layer(xts, f"cin{k}", SP[k], H1, "L1", k,
                                   True, cn)

                if tail_prev is not None:
                    next(tail_prev, None)              # MID(prev)
                h2a = sub_layer(h1, "we2", H1, 128, "L2", 0, True, cn, 0)
                if tail_prev is not None:
                    next(tail_prev, None)              # D2(prev)
                h2b = []
                for mi, (js, jp) in enumerate(_tiles(H2)[1:], start=1):
                    psum = pp.tile([jp, cn], f32, tag="ps")
                    for i in range(H1 // 128):
                        nc.tensor.matmul(
                            psum[:], wsl("we2", i, js, jp), h1[i],
                            start=(i == 0), stop=(i == H1 // 128 - 1),
                        )
                    h2b.append(
                        evac(psum, jp, cn, bcol("L2", 0, js), True,
                             "dve" if mi % 2 == 0 else "act")
                    )
                if tail_prev is not None:
                    next(tail_prev, None)              # out(prev)
                tail_prev = tail_stages(k, g0, cn, h2a + h2b)

            if tail_prev is not None:
                for _ in tail_prev:
                    pass

    nc.compile()
    return nc


def _rearr(a, t, c):
    """[t*128, c] -> [128, t*c] packed K-tile-major per partition."""
    return np.ascontiguousarray(
        a.reshape(t, 128, c).transpose(1, 0, 2).reshape(128, t * c)
    )


def kernel(**inputs):
    global _last_exec_ns
    from concourse.bass_utils import run_bass_kernel_spmd

    x = np.asarray(inputs["x"], dtype=np.float32)
    seq = np.asarray(inputs["seq_lengths"]).astype(np.int64)
    B = x.shape[0]

    Win = np.asarray(inputs["Win"], dtype=np.float32)
    bin_ = np.asarray(inputs["bin_"], dtype=np.float32)
    Wout = np.asarray(inputs["Wout"], dtype=np.float32)
    bout = np.asarray(inputs["bout"], dtype=np.float32)
    We1 = np.asarray(inputs["We1"], dtype=np.float32)
    be1 = np.asarray(inputs["be1"], dtype=np.float32)
    We2 = np.asarray(inputs["We2"], dtype=np.float32)
    be2 = np.asarray(inputs["be2"], dtype=np.float32)
    We3 = np.asarray(inputs["We3"], dtype=np.float32)
    be3 = np.asarray(inputs["be3"], dtype=np.float32)
    Wd1 = np.asarray(inputs["Wd1"], dtype=np.float32)
    bd1 = np.asarray(inputs["bd1"], dtype=np.float32)
    Wd2 = np.asarray(inputs["Wd2"], dtype=np.float32)
    bd2 = np.asarray(inputs["bd2"], dtype=np.float32)
    Wd3 = np.asarray(inputs["Wd3"], dtype=np.float32)
    bd3 = np.asarray(inputs["bd3"], dtype=np.float32)

    idx = [np.nonzero(seq == s)[0] for s in SIZES]
    n_ks = [len(i) for i in idx]
    c_ks = tuple(2 * (-(-n // (2 * N_CORES))) if n > 0 else 0 for n in n_ks)
    R = sum(c_ks)

    out = np.zeros((B, BASE), dtype=np.float32)
    if R == 0:
        return out

    offs = np.cumsum([0] + list(c_ks))[:-1]

    blob_lay, blob_cols = _blob_layout()
    blob = np.zeros((128, blob_cols), dtype=BF16)

    def put(name, w):
        off, t, c = blob_lay[name]
        blob[:, off:off + t * c] = _rearr(w, t, c).astype(BF16)

    M = Wd1 @ We3
    for k in range(5):
        s = SIZES[k]
        C = We1 @ Win[k][:, :s]
        G = Wout[k][:s, :] @ Wd3
        Cp = np.zeros((SP[k], H1), dtype=np.float32)
        Cp[:s] = C.T
        put(f"cin{k}", Cp)
        put(f"gout{k}", np.ascontiguousarray(G.T))
    put("we2", np.ascontiguousarray(We2.T))
    put("mid", np.ascontiguousarray(M.T))
    put("wd2", np.ascontiguousarray(Wd2.T))

    b1 = [We1 @ bin_[k] + be1 for k in range(5)]
    bm = Wd1 @ be3 + bd1
    b2 = [Wout[k][:SIZES[k], :] @ bd3 + bout[k][:SIZES[k]] for k in range(5)]

    bias_cols = _bias_layout()
    bp = np.zeros((128, len(bias_cols)), dtype=np.float32)
    for j, col in enumerate(bias_cols):
        layer, k, start, width = col
        if layer == "L1":
            v = b1[k][start:start + width]
        elif layer == "out":
            v = b2[k][start:start + width]
        elif layer == "L2":
            v = be2[start:start + width]
        elif layer == "MID":
            v = bm[start:start + width]
        else:
            v = bd2[start:start + width]
        bp[: len(v), j] = v

    shared = {"wblob": blob, "biases": bp}

    in_maps = []
    core_rows = []
    for m in range(N_CORES):
        Xc = np.zeros((R, BASE_P), dtype=np.float32)
        rows_info = []
        for k in range(5):
            if c_ks[k] == 0:
                continue
            lo = m * c_ks[k]
            rows = idx[k][lo:lo + c_ks[k]]
            if len(rows):
                Xc[offs[k]:offs[k] + len(rows), :BASE] = x[rows]
            rows_info.append((k, rows, offs[k]))
        in_maps.append(
            {"xT": np.ascontiguousarray(Xc.T).astype(BF16), **shared}
        )
        core_rows.append(rows_info)

    key = (c_ks, R)
    if key not in _prog_cache:
        _prog_cache[key] = _build_program(c_ks, R)
    nc = _prog_cache[key]

    trace = bool(os.environ.get("BASS_TRACE"))
    res = None
    last_exc = None
    had_never = os.environ.get("BASS_NEVER_TRACE")
    for attempt in range(3):
        try:
            res = run_bass_kernel_spmd(
                nc, in_maps, list(range(N_CORES)), trace=trace
            )
            break
        except Exception as exc:
            last_exc = exc
            # retries run without tracing (BASS_TRACE in the env would
            # otherwise re-enable it inside run_bass_kernel_spmd even
            # with trace=False, e.g. when the NTFF hook is unavailable)
            trace = False
            os.environ["BASS_NEVER_TRACE"] = "1"
    if had_never is None:
        os.environ.pop("BASS_NEVER_TRACE", None)
    else:
        os.environ["BASS_NEVER_TRACE"] = had_never
    if res is None:
        raise last_exc
    _last_exec_ns = res.exec_time_ns

    for m in range(N_CORES):
        oT = np.asarray(res.results[m]["outT"]).astype(np.float32)
        for (k, rows, o) in core_rows[m]:
            if len(rows):
                s = SIZES[k]
                out[rows, :s] = oT[:s, o:o + len(rows)].T
    return out


# revision 22
# speedup vs baseline: 1.1610x; 1.1610x over previous
"""Trainium2 Bass kernel for nn_FCAutoEncoder (ragged_sequence).

Strategy:
  * Linear-linear boundaries fold on the host (exact, fp32):
      - per-size input scaler (s_k->1008) + encoder L1 (1008->512)
        -> C_k = We1 @ Win_k  (s_k->512), bias We1 @ bin_k + be1
      - encoder L3 (256->128) + decoder D1 (128->256)
        -> M = Wd1 @ We3      (256->256), bias Wd1 @ be3 + bd1
      - decoder D3 (512->1008) + per-size output scaler (1008->s_k)
        -> G_k = Wout_k @ Wd3 (512->s_k), bias Wout_k @ bd3 + bout_k
    Device work per column: s_k->512->256->256->512->s_k (~1/3 of the
    unfolded FLOPs).
  * Host: bucket rows by seq_length (5 sizes), split each bucket evenly
    across 8 cores (pure data parallel), transpose to feature-major.
  * bf16 everywhere with fp32 PSUM accumulation (rel err ~4e-3 vs fp32
    reference, tolerance 2e-2); LDWEIGHTS (~112ns) hides under the
    ~171ns row stream per matmul.
  * All weights live in ONE packed dram blob, DMAed in a handful of
    slices in first-use order on the Sync queue (per-DMA issue costs
    ~700ns of sequencer time, so fewer/bigger is better); per-unit x
    is one batched DMA, prefetched a unit ahead, also on Sync, AFTER
    the startup-critical weight slices.
  * Unit order [3, 4, 2, 1, 0]: a small bucket first so the PE starts
    on ~0.7MB of DMA instead of ~1.9MB; bucket 4's big weights stream
    during unit 0's compute (cin4 in 4 column-slices so unit 1's
    M-outer matmuls start on the first slice).
  * Two-unit software pipeline: decoder tail of unit i interleaves into
    the encoder head of unit i+1 so the PE never stalls (stalls also
    reset the PE DVFS ramp -> half-clock matmuls).
  * Host: transpose back, scatter rows to original order.
"""
import os
import sys

sys.path.insert(0, "/opt/trn_rl_repo")

import numpy as np
import ml_dtypes

BF16 = ml_dtypes.bfloat16

SIZES = (36, 72, 144, 288, 1008)
SP = (128, 128, 256, 384, 1024)   # SIZES padded to multiples of 128
BASE = 1008
BASE_P = 1024
H1, H2 = 512, 256
N_CORES = 8
MAX_CHUNK = 512
ACT_BUFS = 33
UNIT_ORDER = (3, 4, 2, 1, 0)

_last_exec_ns = None
_prog_cache = {}


def _tiles(n, t=128):
    return [(s, min(t, n - s)) for s in range(0, n, t)]


def _chunks(c, maxn=MAX_CHUNK):
    if c <= 0:
        return []
    assert c % 2 == 0
    half = c // 2
    n = (c + maxn - 1) // maxn
    base, rem = divmod(half, n)
    out, off = [], 0
    for i in range(n):
        sz = 2 * (base + (1 if i < rem else 0))
        out.append((off, sz))
        off += sz
    return out


def _bias_layout():
    cols = []
    for k in range(5):
        for (js, jp) in _tiles(H1):
            cols.append(("L1", k, js, jp))
    for (js, jp) in _tiles(H2):
        cols.append(("L2", 0, js, jp))
    for (js, jp) in _tiles(H2):
        cols.append(("MID", 0, js, jp))
    for (js, jp) in _tiles(H1):
        cols.append(("D2", 0, js, jp))
    for k in range(5):
        for (os_, op) in _tiles(SIZES[k]):
            cols.append(("out", k, os_, op))
    return cols


def _blob_layout():
    """Packed weight blob: name -> (col_offset, n_ktiles, n_cols) in
    first-use order for UNIT_ORDER."""
    k0 = UNIT_ORDER[0]
    entries = [(f"cin{k0}", SP[k0] // 128, H1),
               ("we2", H1 // 128, H2),
               ("mid", H2 // 128, H2),
               ("wd2", H2 // 128, H1),
               (f"gout{k0}", H1 // 128, SIZES[k0])]
    for k in UNIT_ORDER[1:]:
        entries.append((f"cin{k}", SP[k] // 128, H1))
        entries.append((f"gout{k}", H1 // 128, SIZES[k]))
    lay = {}
    off = 0
    for name, t, c in entries:
        lay[name] = (off, t, c)
        off += t * c
    return lay, off


def _build_program(c_ks, R):
    import concourse.bacc as bacc
    import concourse.mybir as mybir
    from concourse import tile

    f32 = mybir.dt.float32
    bf16 = mybir.dt.bfloat16
    AF = mybir.ActivationFunctionType
    ALU = mybir.AluOpType

    bias_cols = _bias_layout()
    bias_idx = {c[:3]: i for i, c in enumerate(bias_cols)}

    def bcol(layer, k, start):
        return bias_idx[(layer, k, start)]

    blob_lay, blob_cols = _blob_layout()

    nc = bacc.Bacc(None, target_bir_lowering=False, debug=False, num_devices=1)

    xT = nc.dram_tensor("xT", [BASE_P, R], bf16, kind="ExternalInput").ap()
    outT = nc.dram_tensor("outT", [BASE, R], bf16, kind="ExternalOutput").ap()
    blobD = nc.dram_tensor("wblob", [128, blob_cols], bf16,
                           kind="ExternalInput").ap()
    biasD = nc.dram_tensor("biases", [128, len(bias_cols)], f32,
                           kind="ExternalInput").ap()

    with tile.TileContext(nc) as tc:
        with (
            tc.tile_pool(name="wp", bufs=1) as wp,
            tc.tile_pool(name="ap", bufs=ACT_BUFS) as apool,
            tc.tile_pool(name="xp", bufs=6) as xpool,
            tc.tile_pool(name="pp", bufs=8, space="PSUM") as pp,
        ):
            bias_t = wp.tile([128, len(bias_cols)], f32, tag="bias")
            blob = wp.tile([128, blob_cols], bf16, tag="blob")

            def wsl(name, i, js, jp):
                off, t, c = blob_lay[name]
                a = off + i * c + js
                return blob[:, a:a + jp]

            def evac(psum, mp, cn, bias_j, relu, eng, out_dt=bf16):
                o = apool.tile([mp, cn], out_dt, tag="act")
                b = bias_t[:mp, bias_j:bias_j + 1]
                if eng == "act":
                    nc.scalar.activation(
                        o[:], psum[:], AF.Relu if relu else AF.Identity, bias=b
                    )
                else:
                    if relu:
                        nc.vector.tensor_scalar(
                            o[:], psum[:], b, 0.0, ALU.add, ALU.max
                        )
                    else:
                        nc.vector.tensor_scalar_add(o[:], psum[:], b)
                return o[:]

            def sub_layer(in_tiles, wname, n_in, n_out, bias_layer, bias_k,
                          relu, cn, eng0=0):
                outs = []
                nkt = n_in // 128
                for mi, (js, jp) in enumerate(_tiles(n_out)):
                    psum = pp.tile([jp, cn], f32, tag="ps")
                    for i in range(nkt):
                        nc.tensor.matmul(
                            psum[:], wsl(wname, i, js, jp), in_tiles[i],
                            start=(i == 0), stop=(i == nkt - 1),
                        )
                    outs.append(
                        evac(psum, jp, cn, bcol(bias_layer, bias_k, js),
                             relu, "dve" if (mi + eng0) % 2 == 0 else "act")
                    )
                return outs

            def load_x(k, g0, cn):
                """One batched DMA on Sync: xT[0:SP_k, ...] -> [128,t,cn]."""
                t = SP[k] // 128
                tl = xpool.tile([128, t, cn], bf16, tag="xin")
                r = xT[0:SP[k], g0:g0 + cn].rearrange("(t p) c -> p t c",
                                                      p=128)
                nc.sync.dma_start(tl[:], r)
                return [tl[:, i, :] for i in range(t)]

            def emit_out(k, g0, cn, d2):
                for oi, (os_, op) in enumerate(_tiles(SIZES[k])):
                    psum = pp.tile([op, cn], f32, tag="ps")
                    for i in range(H1 // 128):
                        nc.tensor.matmul(
                            psum[:], wsl(f"gout{k}", i, os_, op), d2[i],
                            start=(i == 0), stop=(i == H1 // 128 - 1),
                        )
                    ot = evac(psum, op, cn, bcol("out", k, os_),
                              False, "act" if oi % 2 else "dve")
                    nc.sync.dma_start(
                        outT[os_:os_ + op, g0:g0 + cn], ot[:]
                    )

            def tail_stages(k, g0, cn, h2):
                m2 = sub_layer(h2, "mid", H2, H2, "MID", 0, True, cn, 1)
                yield
                d2 = sub_layer(m2, "wd2", H2, H1, "D2", 0, True, cn)
                yield
                emit_out(k, g0, cn, d2)

            offs = {}
            off = 0
            for k in range(5):
                offs[k] = off
                off += c_ks[k]

            units = []
            for k in UNIT_ORDER:
                for (c0, cn) in _chunks(c_ks[k]):
                    units.append((k, offs[k] + c0, cn))

            # ---- startup ----
            k0, g00, cn0 = units[0]
            t0 = SP[k0] // 128
            # first unit's x: one tile per DMA for exact dependencies,
            # on the Scalar HWDGE queue (parallel to Sync's weight DMAs).
            # These issue FIRST: anything ahead of them on the Scalar
            # queue (in particular the ACT-table warm-up, ~1.5us) delays
            # the first matmul.
            h = (t0 + 1) // 2
            x0a = xpool.tile([128, h, cn0], bf16, tag="xin", name="x0a")
            x0b = None
            if t0 - h > 0:
                x0b = xpool.tile([128, t0 - h, cn0], bf16, tag="xin",
                                 name="x0b")
            x0r = xT[0:SP[k0], g00:g00 + cn0].rearrange("(t p) c -> p t c",
                                                        p=128)
            nc.scalar.dma_start(x0a[:], x0r[:, 0:h, :])
            if x0b is not None:
                nc.scalar.dma_start(x0b[:], x0r[:, h:t0, :])
            nc.scalar.dma_start(bias_t[:], biasD[:])
            xts0 = [x0a[:, i, :] for i in range(h)] + \
                   ([x0b[:, i, :] for i in range(t0 - h)] if x0b is not None
                    else [])

            # warm the ACT table (lazy ~1.5us load) off the critical
            # path, after the startup DMAs are queued
            warm_a = wp.tile([1, 2], f32, tag="warm_a")
            warm_b = wp.tile([1, 2], f32, tag="warm_b")
            nc.any.memset(warm_a[:], 0.0)
            nc.scalar.activation(warm_b[:], warm_a[:], AF.Relu)

            # weight blob on Sync, in priority order:
            #  1. first unit's cin, sliced per K-plane (K-outer feed)
            #  2. shared mid layers + first unit's gout
            #  3. second unit's x (big bucket), then its cin in 4
            #     column-slices (M-outer feed) and its gout
            #  4. everything else, then remaining x prefetches
            def blob_dma(a, b):
                nc.sync.dma_start(blob[:, a:b], blobD[:, a:b])

            c0_off, c0_t, c0_c = blob_lay[f"cin{k0}"]
            for i in range(0, c0_t, 2):
                j = min(i + 2, c0_t)
                blob_dma(c0_off + i * c0_c, c0_off + j * c0_c)
            g0_off, g0_t, g0_c = blob_lay[f"gout{k0}"]
            blob_dma(c0_off + c0_t * c0_c, g0_off + g0_t * g0_c)

            xpre = {}
            if len(units) > 1:
                k1, g01, cn1 = units[1]
                xpre[(k1, g01)] = load_x(k1, g01, cn1)
                c1_off, c1_t, c1_c = blob_lay[f"cin{k1}"]
                c1r_s = blob[:, c1_off:c1_off + c1_t * c1_c].rearrange(
                    "p (t c) -> p t c", c=c1_c)
                c1r_d = blobD[:, c1_off:c1_off + c1_t * c1_c].rearrange(
                    "p (t c) -> p t c", c=c1_c)
                for cs in range(0, c1_c, 128):
                    ce = min(cs + 128, c1_c)
                    nc.sync.dma_start(c1r_s[:, :, cs:ce], c1r_d[:, :, cs:ce])
                g1_off, g1_t, g1_c = blob_lay[f"gout{k1}"]
                blob_dma(g1_off, g1_off + g1_t * g1_c)
                if g1_off + g1_t * g1_c < blob_cols:
                    blob_dma(g1_off + g1_t * g1_c, blob_cols)
            for (k, g0, cn) in units[2:]:
                xpre[(k, g0)] = load_x(k, g0, cn)

            tail_prev = None
            for ui, (k, g0, cn) in enumerate(units):
                first = ui == 0
                if first:
                    # K-outer so matmuls start while x/cin stream in
                    nkt = SP[k] // 128
                    jt = _tiles(H1)
                    psums = [pp.tile([jp, cn], f32, tag="ps",
                                     name=f"ps_s0_{mi}")
                             for mi, (js, jp) in enumerate(jt)]
                    for i in range(nkt):
                        for mi, (js, jp) in enumerate(jt):
                            nc.tensor.matmul(
                                psums[mi][:], wsl(f"cin{k}", i, js, jp),
                                xts0[i],
                                start=(i == 0), stop=(i == nkt - 1),
                                skip_group_check=True,
                            )
                    h1 = [
                        evac(psums[mi], jp, cn, bcol("L1", k, js), True,
                             "dve" if mi % 2 == 0 else "act")
                        for mi, (js, jp) in enumerate(jt)
                    ]
                else:
                    xts = xpre.pop((k, g0))
                    h1 = sub_layer(xts, f"cin{k}", SP[k], H1, "L1", k,
                                   True, cn)

                if tail_prev is not None:
                    next(tail_prev, None)              # MID(prev)
                h2a = sub_layer(h1, "we2", H1, 128, "L2", 0, True, cn, 0)
                if tail_prev is not None:
                    next(tail_prev, None)              # D2(prev)
                h2b = []
                for mi, (js, jp) in enumerate(_tiles(H2)[1:], start=1):
                    psum = pp.tile([jp, cn], f32, tag="ps")
                    for i in range(H1 // 128):
                        nc.tensor.matmul(
                            psum[:], wsl("we2", i, js, jp), h1[i],
                            start=(i == 0), stop=(i == H1 // 128 - 1),
                        )
                    h2b.append(
                        evac(psum, jp, cn, bcol("L2", 0, js), True,
                             "dve" if mi % 2 == 0 else "act")
                    )
                if tail_prev is not None:
                    next(tail_prev, None)              # out(prev)
                tail_prev = tail_stages(k, g0, cn, h2a + h2b)

            if tail_prev is not None:
                for _ in tail_prev:
                    pass

    nc.compile()
    return nc


def _rearr(a, t, c):
    """[t*128, c] -> [128, t*c] packed K-tile-major per partition."""
    return np.ascontiguousarray(
        a.reshape(t, 128, c).transpose(1, 0, 2).reshape(128, t * c)
    )


def kernel(**inputs):
    global _last_exec_ns
    from concourse.bass_utils import run_bass_kernel_spmd

    x = np.asarray(inputs["x"], dtype=np.float32)
    seq = np.asarray(inputs["seq_lengths"]).astype(np.int64)
    B = x.shape[0]

    Win = np.asarray(inputs["Win"], dtype=np.float32)
    bin_ = np.asarray(inputs["bin_"], dtype=np.float32)
    Wout = np.asarray(inputs["Wout"], dtype=np.float32)
    bout = np.asarray(inputs["bout"], dtype=np.float32)
    We1 = np.asarray(inputs["We1"], dtype=np.float32)
    be1 = np.asarray(inputs["be1"], dtype=np.float32)
    We2 = np.asarray(inputs["We2"], dtype=np.float32)
    be2 = np.asarray(inputs["be2"], dtype=np.float32)
    We3 = np.asarray(inputs["We3"], dtype=np.float32)
    be3 = np.asarray(inputs["be3"], dtype=np.float32)
    Wd1 = np.asarray(inputs["Wd1"], dtype=np.float32)
    bd1 = np.asarray(inputs["bd1"], dtype=np.float32)
    Wd2 = np.asarray(inputs["Wd2"], dtype=np.float32)
    bd2 = np.asarray(inputs["bd2"], dtype=np.float32)
    Wd3 = np.asarray(inputs["Wd3"], dtype=np.float32)
    bd3 = np.asarray(inputs["bd3"], dtype=np.float32)

    idx = [np.nonzero(seq == s)[0] for s in SIZES]
    n_ks = [len(i) for i in idx]
    c_ks = tuple(2 * (-(-n // (2 * N_CORES))) if n > 0 else 0 for n in n_ks)
    R = sum(c_ks)

    out = np.zeros((B, BASE), dtype=np.float32)
    if R == 0:
        return out

    offs = np.cumsum([0] + list(c_ks))[:-1]

    blob_lay, blob_cols = _blob_layout()
    blob = np.zeros((128, blob_cols), dtype=BF16)

    def put(name, w):
        off, t, c = blob_lay[name]
        blob[:, off:off + t * c] = _rearr(w, t, c).astype(BF16)

    M = Wd1 @ We3
    for k in range(5):
        s = SIZES[k]
        C = We1 @ Win[k][:, :s]
        G = Wout[k][:s, :] @ Wd3
        Cp = np.zeros((SP[k], H1), dtype=np.float32)
        Cp[:s] = C.T
        put(f"cin{k}", Cp)
        put(f"gout{k}", np.ascontiguousarray(G.T))
    put("we2", np.ascontiguousarray(We2.T))
    put("mid", np.ascontiguousarray(M.T))
    put("wd2", np.ascontiguousarray(Wd2.T))

    b1 = [We1 @ bin_[k] + be1 for k in range(5)]
    bm = Wd1 @ be3 + bd1
    b2 = [Wout[k][:SIZES[k], :] @ bd3 + bout[k][:SIZES[k]] for k in range(5)]

    bias_cols = _bias_layout()
    bp = np.zeros((128, len(bias_cols)), dtype=np.float32)
    for j, col in enumerate(bias_cols):
        layer, k, start, width = col
        if layer == "L1":
            v = b1[k][start:start + width]
        elif layer == "out":
            v = b2[k][start:start + width]
        elif layer == "L2":
            v = be2[start:start + width]
        elif layer == "MID":
            v = bm[start:start + width]
        else:
            v = bd2[start:start + width]
        bp[: len(v), j] = v

    shared = {"wblob": blob, "biases": bp}

    in_maps = []
    core_rows = []
    for m in range(N_CORES):
        Xc = np.zeros((R, BASE_P), dtype=np.float32)
        rows_info = []
        for k in range(5):
            if c_ks[k] == 0:
                continue
            lo = m * c_ks[k]
            rows = idx[k][lo:lo + c_ks[k]]
            if len(rows):
                Xc[offs[k]:offs[k] + len(rows), :BASE] = x[rows]
            rows_info.append((k, rows, offs[k]))
        in_maps.append(
            {"xT": np.ascontiguousarray(Xc.T).astype(BF16), **shared}
        )
        core_rows.append(rows_info)

    key = (c_ks, R)
    if key not in _prog_cache:
        _prog_cache[key] = _build_program(c_ks, R)
    nc = _prog_cache[key]

    trace = bool(os.environ.get("BASS_TRACE"))
    res = None
    last_exc = None
    had_never = os.environ.get("BASS_NEVER_TRACE")
    for attempt in range(3):
        try:
            res = run_bass_kernel_spmd(
                nc, in_maps, list(range(N_CORES)), trace=trace
            )
            break
        except Exception as exc:
            last_exc = exc
            # retries run without tracing (BASS_TRACE in the env would
            # otherwise re-enable it inside run_bass_kernel_spmd even
            # with trace=False, e.g. when the NTFF hook is unavailable)
            trace = False
            os.environ["BASS_NEVER_TRACE"] = "1"
    if had_never is None:
        os.environ.pop("BASS_NEVER_TRACE", None)
    else:
        os.environ["BASS_NEVER_TRACE"] = had_never
    if res is None:
        raise last_exc
    _last_exec_ns = res.exec_time_ns

    for m in range(N_CORES):
        oT = np.asarray(res.results[m]["outT"]).astype(np.float32)
        for (k, rows, o) in core_rows[m]:
            if len(rows):
                s = SIZES[k]
                out[rows, :s] = oT[:s, o:o + len(rows)].T
    return out
